# revision 94
# baseline (speedup 1.0000x reference)
"""Disentangled multi-head attention (DeBERTa-style) on 8 Trainium2 NeuronCores.

Sharding: core c -> batch b = c // 4, head group g = c % 4 (4 of 16 heads).
Each core computes its 4 heads end-to-end (column-parallel QKV projections,
attention, row-parallel slice of the output projection); the host sums the
4 partial outputs per batch in fp32 and adds the bias terms.

Math folds (exact up to bf16 rounding):
  - scores = (q_c.(k_c+k_p) + q_p.k_c) * s as ONE K=128 matmul per tile with
    per-head channel layout [qc*s; qp*s] vs [kc+kp; kc] (scale folded into
    weights).  ODD heads use the flipped layout [qp; qc] vs [kc; kcp] so that
    head-PAIR-packed projection matmuls (full 128-wide stationary tiles, no
    zero padding) produce partition-aligned PSUM->SBUF copies.
  - gate: Wg*(1/s) replicated across 128 stationary columns -> matmul with
    qc*s gives the partition-broadcast pre-activation; Sigmoid on ACT.
  - gate*spatial_bias accumulated into the score PSUM via 4 sub-matmuls
    lhsT=sb[qblock, kblock] (q-major spatial bias, straight from HBM) and
    rhs=diag(gate[qblock]) built on DVE as ident*gb.  The gate multiply
    rides the PE array; no elementwise gate*bias pass exists anywhere.
  - softmax without max-subtraction (scores bounded ~+-8, fp32-exp safe).
  - row-sums from a ones-column matmul sharing the et stationary; ctx
    normalization is a single tensor_scalar divide per (head, qblock).
  - transposes packed 2 heads per 128x128 tile; bv and bo folded on host
    (softmax rows sum to 1 when mask is all-True).

Scheduling: all hot inputs are host-repacked into few, wide DMAs (HWDGE
fixed cost is 625ns/transfer); x/sb streams are prefetched one phase ahead
and sized just-in-time; PE is pre-warmed on a scratch tile to ramp the
p-state while the first DMAs land.
"""

import sys

sys.path.insert(0, "/opt/trn_rl_repo")

from contextlib import ExitStack

import numpy as np
import ml_dtypes

import concourse.bass as bass
from concourse import mybir, masks
from concourse.tile import TileContext
from concourse.bass_utils import run_bass_kernel_spmd

BF16 = ml_dtypes.bfloat16
FP8 = ml_dtypes.float8_e4m3

B, L, D = 2, 2048, 1024
H = 16
HK = 64          # head dim
NCORES = 8
HPC = 4          # heads per core
CS = HPC * HK    # channels per core = 256
NJ = L // 128    # 16 key/token blocks
NCH = L // 512   # 4 query chunks
KB_D = D // 128  # 8 contraction blocks for a 1024-deep dim
KB_2D = 2 * KB_D
SCALE = float(1.0 / np.sqrt(HK))
NWARM = 34       # PE p-state warmup matmuls

_IDENT = mybir.ActivationFunctionType.Identity

_FP32 = mybir.dt.float32
_BF16 = mybir.dt.bfloat16
_FP8 = mybir.dt.float8e4
_EXP = mybir.ActivationFunctionType.Exp
_SIG = mybir.ActivationFunctionType.Sigmoid
_DROW = mybir.MatmulPerfMode.DoubleRow


def _split_multiwaits(nc, skip_opcodes=()):
    """This walrus build encodes at most one sync-wait per TPB instruction.
    Tile attaches several; hoist the extras onto same-engine NoOps placed
    immediately before the instruction (engines are in-order, so semantics
    are preserved)."""
    nsplit = 0
    for fn in nc.m.functions:
        for blk in fn.blocks:
            insts = blk.instructions
            out = []
            for inst in insts:
                si = inst.sync_info
                waits = list(si.on_wait) if si is not None and si.on_wait else []
                if len(waits) > 1 and inst.opcode not in skip_opcodes:
                    si.on_wait = waits[-1:]
                    for i, w in enumerate(waits[:-1]):
                        nop = mybir.InstNoOp(name=f"{inst.name}-w{i}",
                                             ins=[], outs=[])
                        nop.engine = inst.engine
                        nop.sync_info = type(si)(on_wait=[w], on_update=[])
                        out.append(nop)
                    nsplit += 1
                out.append(inst)
            if len(out) != len(insts):
                blk.instructions = out
    return nsplit


DEBUG = False


def build_nc():
    """Emit the per-core BIR (identical on all 8 cores; data differs)."""
    nc = bass.Bass()
    if DEBUG:
        dbg_qc = nc.dram_tensor("dbg_qc", [4 * 128, L], _BF16,
                                kind="ExternalOutput")
        dbg_kc = nc.dram_tensor("dbg_kc", [4 * 128, L], _BF16,
                                kind="ExternalOutput")
        dbg_pcv = nc.dram_tensor("dbg_pcv", [2 * 128, 512], _BF16,
                                 kind="ExternalOutput")
        dbg_pcs = nc.dram_tensor("dbg_pcs", [128, 16], _BF16,
                                 kind="ExternalOutput")
        dbg_vb = nc.dram_tensor("dbg_vb", [128, CS], _BF16,
                                kind="ExternalOutput")
        dbg_gb = nc.dram_tensor("dbg_gb", [4 * 128, 512], _BF16,
                                kind="ExternalOutput")

    # host-repacked wide layouts (see kernel() for the packing)
    xqr = nc.dram_tensor("xqr", [128, NCH * 8192], _BF16, kind="ExternalInput")
    xkk = nc.dram_tensor("xkk", [2 * D, L], _BF16, kind="ExternalInput")
    xvr = nc.dram_tensor("xvr", [128, 16384], _BF16, kind="ExternalInput")
    sbq = nc.dram_tensor("sbq", [L, L], _BF16, kind="ExternalInput")
    wqp = nc.dram_tensor("wqp", [128, 4096], _BF16, kind="ExternalInput")
    wkc2 = nc.dram_tensor("wkc2", [128, 4096], _BF16, kind="ExternalInput")
    wkk2 = nc.dram_tensor("wkk2", [128, 2048], _BF16, kind="ExternalInput")
    wvr = nc.dram_tensor("wvr", [128, 2048], _BF16, kind="ExternalInput")
    wg8 = nc.dram_tensor("wg8", [128, 128], _BF16, kind="ExternalInput")
    wo = nc.dram_tensor("wo", [CS, D], _BF16, kind="ExternalInput")
    pb = nc.dram_tensor("pb", [128, 8], _FP32, kind="ExternalInput")
    g0 = nc.dram_tensor("g0", [128, 1], _FP32, kind="ExternalInput")
    outT = nc.dram_tensor("outT", [D, L], _BF16, kind="ExternalOutput")

    with TileContext(nc) as tc, ExitStack() as top:
        pool = lambda **kw: top.enter_context(tc.tile_pool(**kw))

        const_pool = pool(name="const", bufs=1)
        w_pool = pool(name="w", bufs=1)
        bias_pool = pool(name="bias", bufs=1)
        qk_pool = pool(name="qkres", bufs=1)
        v_pool = pool(name="vres", bufs=1)
        sb_pool = pool(name="sb", bufs=8)      # streamed ring
        x_pool = pool(name="xin", bufs=1)      # per-tag rings set on tile()
        gb_pool = pool(name="gb", bufs=1)
        dg_pool = pool(name="dg", bufs=1)
        e_pool = pool(name="et", bufs=3)
        csb_pool = pool(name="csb", bufs=1)
        cta_pool = pool(name="cta", bufs=1)
        oute_pool = pool(name="oute", bufs=3)

        scr = const_pool.tile([128, 128], _BF16, tag="scr", name="scr")
        nc.gpsimd.memset(scr[:], 0.0)
        ident = const_pool.tile([128, 128], _BF16, tag="ident", name="ident")
        masks.make_identity(nc, ident[:])
        ident32 = const_pool.tile([128, 128], _FP32, tag="id32", name="ident32")
        masks.make_identity(nc, ident32[:])
        ones_t = const_pool.tile([128, 1], _BF16, tag="ones", name="ones")
        nc.gpsimd.memset(ones_t[:], 1.0)

        # ---- DMA issue order = consumption order (HWDGE is in-order) -----
        wv_t = w_pool.tile([128, 2048], _BF16, tag="wv", name="wvt")
        nc.sync.dma_start(wv_t[:], wvr[:, :])
        xv_t = []
        for i in range(8):  # (half, kb-pair) tiles, consumption order
            t = x_pool.tile([128, 2048], _BF16, tag="xv", name=f"xv{i}",
                            bufs=8)
            nc.sync.dma_start(t[:], xvr[:, i * 2048:(i + 1) * 2048])
            xv_t.append(t)
        wkc_t = w_pool.tile([128, 4096], _BF16, tag="wkc", name="wkct")
        nc.sync.dma_start(wkc_t[:], wkc2[:, :])
        wkk_t = w_pool.tile([128, 2048], _BF16, tag="wkk", name="wkkt")
        nc.sync.dma_start(wkk_t[:], wkk2[:, :])
        wg8_t = const_pool.tile([128, 128], _BF16, tag="wg8", name="wg8t")
        nc.sync.dma_start(wg8_t[:], wg8[:, :])
        pb_t = bias_pool.tile([128, 8], _FP32, tag="pb", name="pbt")
        nc.sync.dma_start(pb_t[:], pb[:, :])
        g0_t = bias_pool.tile([128, 1], _FP32, tag="g0", name="g0t")
        nc.sync.dma_start(g0_t[:], g0[:, :])

        _cb_tog = [0]

        def copy_bias(dst, src, bias_ap, on_dve):
            """PSUM->SBUF copy with per-partition bias add, on DVE or ACT."""
            if on_dve is None:  # alternate
                _cb_tog[0] ^= 1
                on_dve = bool(_cb_tog[0])
            if on_dve:
                nc.vector.tensor_scalar_add(dst, src, bias_ap)
            else:
                nc.scalar.activation(dst, src, _IDENT, bias=bias_ap)



        def wkc_s(kb, pcol):
            return wkc_t[:, kb * 256 + pcol.start:kb * 256 + pcol.stop]

        def wkk_s(kb, pcol):
            return wkk_t[:, kb * 256 + pcol.start:kb * 256 + pcol.stop]

        qcat = [qk_pool.tile([128, L], _BF16, tag=f"qcat{h}", name=f"qcat{h}")
                for h in range(HPC)]
        kcat = [qk_pool.tile([128, L], _BF16, tag=f"kcat{h}", name=f"kcat{h}")
                for h in range(HPC)]
        vb4 = [None] * NJ
        sbq_t = [None] * NJ
        xq_t = [None] * NCH
        gb_sb = {}

        def load_sbq(rows):
            for r in rows:
                t = sb_pool.tile([128, L], _BF16, tag="sbq", name=f"sbq{r}")
                nc.sync.dma_start(t[:], sbq[r * 128:(r + 1) * 128, :])
                sbq_t[r] = t

        def load_xq(ch):
            """Two wide DMAs for p2q(ch): q-side and p-side kb strips."""
            tq = x_pool.tile([128, 4096], _BF16, tag="xq", name=f"xq{ch}",
                             bufs=4)
            nc.sync.dma_start(tq[:], xqr[:, ch * 8192:ch * 8192 + 4096])
            tp = x_pool.tile([128, 4096], _BF16, tag="xq", name=f"xp{ch}",
                             bufs=4)
            nc.sync.dma_start(tp[:], xqr[:, ch * 8192 + 4096:(ch + 1) * 8192])
            xq_t[ch] = (tq, tp)

        # ---- PE warmup on scratch: ramp p-state while DMAs land ---------
        with tc.tile_pool(name="ps_warm", bufs=1, space="PSUM") as warm_pool:
            wmt = warm_pool.tile([128, 128], _FP32, tag="warm", name="warm")
            for i in range(NWARM):
                nc.tensor.matmul(wmt[:], scr[:], scr[:], start=True, stop=True)

        # ---- P1: v projection, kb-pair tiles, bf16 (Wv precision is
        # load-bearing: its quantization error does not average out) -------
        with tc.tile_pool(name="ps_v", bufs=8, space="PSUM") as psv_pool:
            for half in range(2):
                psv = [psv_pool.tile([128, CS], _FP32, tag="ps_v",
                                     name=f"psv{half}_{i}") for i in range(8)]
                for kbp in range(4):
                    xt = xv_t[half * 4 + kbp]
                    for kk in range(2):
                        for i in range(8):
                            nc.tensor.matmul(
                                psv[i][:],
                                xt[:, kk * 1024 + i * 128:
                                   kk * 1024 + (i + 1) * 128],
                                wv_t[:, (kbp * 2 + kk) * 256:
                                     (kbp * 2 + kk + 1) * 256],
                                start=(kbp == 0 and kk == 0),
                                stop=(kbp == 3 and kk == 1))
                for i in range(8):
                    tb = half * 8 + i
                    vb = v_pool.tile([128, CS], _BF16, tag=f"vb{tb}",
                                     name=f"vb{tb}")
                    nc.vector.tensor_copy(vb[:], psv[i][:])
                    vb4[tb] = vb

        # ---- P2kk: k-side projections, head-pair packed, kb-outer -------
        # kcat[even] = [kc+kp ; kc], kcat[odd] = [kc ; kc+kp]
        with tc.tile_pool(name="ps_k", bufs=8, space="PSUM") as psk_pool:
            for pc in range(2):  # ch pair (0,1), (2,3)
                ps1 = [[psk_pool.tile([128, 512], _FP32, tag="psk",
                                      name=f"k1_{pc}_{pr}_{i}")
                        for i in range(2)] for pr in range(2)]
                ps2 = [[psk_pool.tile([128, 512], _FP32, tag="psk",
                                      name=f"k2_{pc}_{pr}_{i}")
                        for i in range(2)] for pr in range(2)]
                for kb in range(KB_2D):
                    t = x_pool.tile([128, 1024], _BF16, tag="xkk",
                                    name=f"xkk{pc}_{kb}", bufs=6)
                    nc.sync.dma_start(
                        t[:], xkk[kb * 128:(kb + 1) * 128,
                                  pc * 1024:(pc + 1) * 1024])
                    for pr in range(2):
                        pcol = slice(pr * 128, (pr + 1) * 128)
                        for i in range(2):
                            isl = slice(i * 512, (i + 1) * 512)
                            nc.tensor.matmul(
                                ps1[pr][i][:], wkc_s(kb, pcol), t[:, isl],
                                start=(kb == 0), stop=(kb == KB_2D - 1))
                            if kb < KB_D:
                                nc.tensor.matmul(
                                    ps2[pr][i][:], wkk_s(kb, pcol), t[:, isl],
                                    start=(kb == 0), stop=(kb == KB_D - 1))
                if pc == 0:
                    # q-side weights + first chunk streams ride between the
                    # two xkk sections
                    wqp_t = w_pool.tile([128, 4096], _BF16, tag="wqp",
                                        name="wqpt")
                    nc.sync.dma_start(wqp_t[:], wqp[:, :])
                for pr in range(2):
                    he, ho = 2 * pr, 2 * pr + 1
                    c = 4 + 2 * pr
                    for i in range(2):
                        csl = slice((2 * pc + i) * 512, (2 * pc + i + 1) * 512)
                        copy_bias(kcat[he][0:64, csl], ps1[pr][i][0:64, :],
                                  pb_t[0:64, c:c + 1], i == 0)
                        copy_bias(kcat[ho][64:128, csl], ps1[pr][i][64:128, :],
                                  pb_t[64:128, c:c + 1], i == 0)
                        copy_bias(kcat[ho][0:64, csl], ps2[pr][i][0:64, :],
                                  pb_t[0:64, c + 1:c + 2], i == 1)
                        copy_bias(kcat[he][64:128, csl], ps2[pr][i][64:128, :],
                                  pb_t[64:128, c + 1:c + 2], i == 1)

        load_xq(0)
        load_sbq(range(0, 4))
        wo_t = w_pool.tile([128, D], _BF16, tag="wo", name="wot")
        nc.sync.dma_start(wo_t[:], wo[0:128, :])
        wo_t2 = w_pool.tile([128, D], _BF16, tag="wo2", name="wot2")
        nc.sync.dma_start(wo_t2[:], wo[128:256, :])
        wo_ts = [wo_t, wo_t2]

        # ---- shared PSUM pools (8 banks exactly) -------------------------
        # ps2: [128,1024] doubles spanning TWO banks; each half gets its own
        # bank so the two halves' accumulation groups are independent
        # (HW start=True resets per-bank).  3-deep ring: scores pipeline 2
        # ahead of the exp; transposes and the out-projection ride the same
        # ring.  A processes one head-pair per pass so ctx accumulation
        # needs a single bank (+1 for rowsums).
        ps2_pool = pool(name="ps2", bufs=3, space="PSUM")
        pcv_pool = pool(name="ps_cv", bufs=1, space="PSUM")   # ctx accum
        pcs_pool = pool(name="ps_cs", bufs=1, space="PSUM")   # rowsums

        # ---- per-chunk pipeline ------------------------------------------
        def p2q_pair(ch, pr):
            """q-side projections for chunk ch, head pair pr, followed by the
            pair's gate/diag so A can start as soon as pair 0 lands."""
            csl = slice(ch * 512, (ch + 1) * 512)
            tq, tp = xq_t[ch]
            d = ps2_pool.tile([128, 1024], _FP32, tag="ps2", name=f"q{ch}{pr}")
            mq, mp = d[:, 0:512], d[:, 512:1024]
            for kb in range(KB_D):
                st, sp = (kb == 0), (kb == KB_D - 1)
                x = tq[:, kb * 512:(kb + 1) * 512]
                nc.tensor.matmul(mq, wqp_t[:, kb * 512 + pr * 128:
                                           kb * 512 + (pr + 1) * 128],
                                 x[:, :], start=st, stop=sp)
                x = tp[:, kb * 512:(kb + 1) * 512]
                nc.tensor.matmul(mp, wqp_t[:, kb * 512 + 256 + pr * 128:
                                           kb * 512 + 256 + (pr + 1) * 128],
                                 x[:, :], start=st, stop=sp)
            he, ho = 2 * pr, 2 * pr + 1
            c = 2 * pr
            on_dve = (pr == 0) if ch > 0 else None  # ch0: ACT is idle
            copy_bias(qcat[he][0:64, csl], d[0:64, 0:512],
                      pb_t[0:64, c:c + 1], on_dve)
            copy_bias(qcat[ho][64:128, csl], d[64:128, 0:512],
                      pb_t[64:128, c:c + 1], on_dve)
            copy_bias(qcat[ho][0:64, csl], d[0:64, 512:1024],
                      pb_t[0:64, c + 1:c + 2], on_dve)
            copy_bias(qcat[he][64:128, csl], d[64:128, 512:1024],
                      pb_t[64:128, c + 1:c + 2], on_dve)
            # gate for both heads of the pair: REUSE the projection double
            # (its copies are done), one Sigmoid instruction over [128,1024]
            for hh, h in enumerate((he, ho)):
                rows = slice(0, 64) if h % 2 == 0 else slice(64, 128)
                nc.tensor.matmul(d[:, hh * 512:(hh + 1) * 512],
                                 wg8_t[rows, :], qcat[h][rows, csl],
                                 start=True, stop=True)
            g = gb_pool.tile([128, 1024], _BF16, tag=f"gb{pr}",
                             name=f"gb{pr}_{ch}")
            nc.scalar.activation(g[:], d[:], _SIG, bias=g0_t[:, 0:1])
            gb_sb[pr] = g
            dg = []
            for hh, h in enumerate((he, ho)):
                ds = []
                for sb in range(4):
                    d2 = dg_pool.tile([128, 128], _BF16, tag=f"dg{h}_{sb}",
                                      name=f"dg{h}_{sb}_{ch}")
                    nc.vector.tensor_mul(
                        d2[:], ident[:],
                        g[:, hh * 512 + sb * 128:hh * 512 + (sb + 1) * 128])
                    ds.append(d2)
                dg.append(ds)
            return dg

        def attn_pass(ch, hp, dg, pcs, fillers=()):
            """Score + gated spatial bias + exp + ctx/rowsum accumulation
            for ONE head pair; `fillers` are small PE work units emitted one
            per j iteration to absorb the ACT exp-lag bubbles."""
            if hp == 0 and ch + 1 < NCH:
                load_xq(ch + 1)
                load_sbq(range(4 * (ch + 1), 4 * (ch + 2)))
            dsl = slice(ch * 512, (ch + 1) * 512)
            pcv = pcv_pool.tile([128, 512], _FP32, tag="ps_cv",
                                name=f"pcv{hp}_{ch}")
            nc.tensor.matmul(pcv[:], scr[:], qcat[0][:, dsl],
                             start=True, stop=False)
            if hp == 0:
                nc.tensor.matmul(pcs[:], scr[:],
                                 qcat[0][:, ch * 512:ch * 512 + 16],
                                 start=True, stop=False)
            if DEBUG and ch == 0:
                for pr in range(2):
                    dt_ = e_pool.tile([128, 1024], _BF16, tag="dbg",
                                      name=f"dbgg{pr}", bufs=2)
                    nc.vector.tensor_copy(dt_[:], gb_sb[pr][:])
                    nc.sync.dma_start(
                        dbg_gb[2 * pr * 128:(2 * pr + 1) * 128, :],
                        dt_[:, 0:512])
                    nc.sync.dma_start(
                        dbg_gb[(2 * pr + 1) * 128:(2 * pr + 2) * 128, :],
                        dt_[:, 512:1024])
            fillers = list(fillers)
            for j in range(NJ):
                if j > 0 and fillers:
                    fillers.pop(0)()
                jsl = slice(j * 128, (j + 1) * 128)
                d = ps2_pool.tile([128, 1024], _FP32, tag="ps2", name="pss")
                for hh in range(2):
                    h = 2 * hp + hh
                    hof = hh * 512
                    nc.tensor.matmul(d[:, hof:hof + 512],
                                     kcat[h][:, jsl], qcat[h][:, dsl],
                                     start=True, stop=False)
                    for sb in range(4):
                        nc.tensor.matmul(
                            d[:, hof + sb * 128:hof + (sb + 1) * 128],
                            sbq_t[ch * 4 + sb][:, jsl],
                            dg[h - 2 * hp][sb][:],
                            start=False, stop=True)
                et = e_pool.tile([128, 1024], _BF16, tag="et", name="ett")
                nc.scalar.activation(et[:], d[:], _EXP)
                for hh in range(2):
                    h = 2 * hp + hh
                    for s in range(4):
                        esl = et[:, hh * 512 + s * 128:
                                 hh * 512 + (s + 1) * 128]
                        nc.tensor.matmul(
                            pcv[:, hh * 256 + s * 64:hh * 256 + (s + 1) * 64],
                            esl, vb4[j][:, h * 64:(h + 1) * 64],
                            start=False, stop=(j == NJ - 1))
                        nc.tensor.matmul(
                            pcs[:, h * 4 + s:h * 4 + s + 1],
                            esl, ones_t[:],
                            start=False, stop=(j == NJ - 1))
            for u in fillers:  # flush any unconsumed work units
                u()
            return pcv

        def norm_div(ch, pr, pcv, pcs):
            """Normalize one pair's ctx by its rowsums; frees pcv's bank."""
            if DEBUG and ch == 0:
                dt_ = e_pool.tile([128, 512], _BF16, tag="dbg",
                                  name=f"dbgv{pr}", bufs=4)
                nc.vector.tensor_copy(dt_[:], pcv[:])
                nc.sync.dma_start(dbg_pcv[pr * 128:(pr + 1) * 128, :], dt_[:])
                if pr == 1:
                    dts = e_pool.tile([128, 16], _BF16, tag="dbgs", name="dbgs")
                    nc.vector.tensor_copy(dts[:], pcs[:])
                    nc.sync.dma_start(dbg_pcs[:, :], dts[:])
            inv = csb_pool.tile([128, 8], _FP32, tag=f"inv{pr}",
                                name=f"inv{pr}_{ch}")
            nc.vector.reciprocal(inv[:], pcs[:, pr * 8:(pr + 1) * 8])
            csb2 = {}
            for s in range(4):
                t = csb_pool.tile([128, 128], _FP32, tag=f"cs{pr}_{s}",
                                  name=f"cs{pr}_{s}_{ch}")
                for hh in range(2):
                    if (s + hh) % 2 == 0:
                        nc.vector.tensor_scalar_mul(
                            t[:, hh * 64:(hh + 1) * 64],
                            pcv[:, hh * 256 + s * 64:hh * 256 + (s + 1) * 64],
                            inv[:, hh * 4 + s:hh * 4 + s + 1])
                    else:
                        nc.scalar.activation(
                            t[:, hh * 64:(hh + 1) * 64],
                            pcv[:, hh * 256 + s * 64:hh * 256 + (s + 1) * 64],
                            mybir.ActivationFunctionType.Copy,
                            scale=inv[:, hh * 4 + s:hh * 4 + s + 1])
                csb2[s] = t
            return csb2

        def cta_tile(ch, pr):
            return cta_pool.tile([128, 512], _BF16, tag=f"cta{pr}",
                                 name=f"cta{pr}_{ch}")

        def t_units(csb2, cta):
            """Transposes for one pair, 2 per ring double (one per bank)."""
            units = []

            def t_unit(s2):
                def go():
                    d = ps2_pool.tile([128, 1024], _FP32, tag="ps2",
                                      name="ptt")
                    for hh in range(2):
                        s = 2 * s2 + hh
                        nc.tensor.matmul(d[:, hh * 512:hh * 512 + 128],
                                         csb2[s][:], ident32[:],
                                         is_transpose=True)
                    for hh in range(2):
                        s = 2 * s2 + hh
                        nc.vector.tensor_copy(
                            cta[:, s * 128:(s + 1) * 128],
                            d[:, hh * 512:hh * 512 + 128])
                return go

            for s2 in range(2):
                units.append(t_unit(s2))
            return units

        def o_units(ch, cta01):
            units = []

            def o_unit(op):
                def go():
                    d = ps2_pool.tile([128, 1024], _FP32, tag="ps2",
                                      name=f"o{ch}{op}")
                    for hh in range(2):
                        ob = 2 * op + hh
                        for kb in range(2):
                            nc.tensor.matmul(
                                d[:, hh * 512:(hh + 1) * 512],
                                wo_ts[kb][:, ob * 128:(ob + 1) * 128],
                                cta01[kb][:], start=(kb == 0), stop=(kb == 1))
                    ot = oute_pool.tile([128, 1024], _BF16, tag="ot",
                                        name="ott", bufs=4)
                    if op % 2 == 0:
                        nc.vector.tensor_copy(ot[:], d[:])
                    else:
                        nc.scalar.copy(ot[:], d[:])
                    for hh in range(2):
                        ob = 2 * op + hh
                        nc.sync.dma_start(
                            outT[ob * 128:(ob + 1) * 128,
                                 ch * 512:(ch + 1) * 512],
                            ot[:, hh * 512:(hh + 1) * 512])
                return go

            for op in range(KB_D // 2):
                units.append(o_unit(op))
            return units

        # pipeline: passA(ch) absorbs prev chunk's pair-1 transposes + O;
        # passB(ch) absorbs this chunk's pair-0 transposes.
        prev = None          # (ch, pcv_pair1, pcs, cta_pair0)
        for ch in range(NCH):
            dg0 = p2q_pair(ch, 0)
            fillA = []
            if prev is not None:
                pch, ppcv1, ppcs, pcta0 = prev
                csb2p1 = norm_div(pch, 1, ppcv1, ppcs)
            dg1 = p2q_pair(ch, 1)
            if prev is not None:
                pcta1 = cta_tile(pch, 1)
                fillA = t_units(csb2p1, pcta1) + o_units(pch, [pcta0, pcta1])
            pcs = pcs_pool.tile([128, 16], _FP32, tag="pcs", name=f"pcs{ch}")
            pcv0 = attn_pass(ch, 0, dg0, pcs, fillA)
            csb2a = norm_div(ch, 0, pcv0, pcs)
            cta0 = cta_tile(ch, 0)
            pcv1 = attn_pass(ch, 1, dg1, pcs, t_units(csb2a, cta0))
            prev = (ch, pcv1, pcs, cta0)
        pch, ppcv1, ppcs, pcta0 = prev
        csb2p1 = norm_div(pch, 1, ppcv1, ppcs)
        pcta1 = cta_tile(pch, 1)
        for u in t_units(csb2p1, pcta1) + o_units(pch, [pcta0, pcta1]):
            u()

        if DEBUG:
            for h in range(HPC):
                for half in range(2):
                    csl = slice(half * 1024, (half + 1) * 1024)
                    dq = e_pool.tile([128, 1024], _BF16, tag="dbgq",
                                     name=f"dbq{h}{half}", bufs=2)
                    nc.vector.tensor_copy(dq[:], qcat[h][:, csl])
                    nc.sync.dma_start(dbg_qc[h * 128:(h + 1) * 128, csl], dq[:])
                    dk = e_pool.tile([128, 1024], _BF16, tag="dbgq",
                                     name=f"dbk{h}{half}", bufs=2)
                    nc.vector.tensor_copy(dk[:], kcat[h][:, csl])
                    nc.sync.dma_start(dbg_kc[h * 128:(h + 1) * 128, csl], dk[:])
            dv = e_pool.tile([128, CS], _BF16, tag="dbgvb", name="dbvb")
            nc.vector.tensor_copy(dv[:], vb4[0][:])
            nc.sync.dma_start(dbg_vb[:, :], dv[:])

    _split_multiwaits(nc)
    return nc


_NC_CACHE = {}


def _get_nc():
    if "nc" not in _NC_CACHE:
        _NC_CACHE["nc"] = build_nc()
    return _NC_CACHE["nc"]


def _np_reference(k, v, q, mask, spatial_bias, pos_k, pos_q,
                  Wk, bk, Wv, bv, Wq, bq, Wpk, bpk, Wpq, bpq, Wo, bo, Wg, bg):
    """Slow numpy fallback (only if mask is not all-True)."""
    def lin(x, W, b):
        return x @ W.T + b

    def split(x):
        return x.reshape(B, L, H, -1).transpose(0, 2, 1, 3)

    k_c, v_c, q_c = split(lin(k, Wk, bk)), split(lin(v, Wv, bv)), split(lin(q, Wq, bq))
    k_p, q_p = split(lin(pos_k, Wpk, bpk)), split(lin(pos_q, Wpq, bpq))
    scores = (np.einsum("bhqd,bhkd->bhqk", q_c, k_c)
              + np.einsum("bhqd,bhkd->bhqk", q_c, k_p)
              + np.einsum("bhqd,bhkd->bhqk", q_p, k_c)) * SCALE
    gate = 1.0 / (1.0 + np.exp(-(q_c @ Wg.T + bg)))
    scores = scores + gate * spatial_bias
    scores = np.where(mask[:, None, :, :], scores, -np.inf)
    scores = scores - scores.max(-1, keepdims=True)
    e = np.exp(scores)
    attn = e / e.sum(-1, keepdims=True)
    ctx = np.einsum("bhqk,bhkd->bhqd", attn, v_c)
    ctx = ctx.transpose(0, 2, 1, 3).reshape(B, L, D)
    return lin(ctx, Wo, bo).astype(np.float32)


def _hstripe(x):
    """[N*128, M] -> [128, N*M]: stack 128-row blocks horizontally."""
    n = x.shape[0] // 128
    return np.concatenate([x[i * 128:(i + 1) * 128] for i in range(n)], axis=1)


def _wv8(wvcs):
    """[1024, 256] -> [128, 2048] fp8 DoubleRow packing by kb-pairs."""
    blocks = []
    for kbp in range(4):
        blocks.append(np.stack(
            [wvcs[(2 * kbp) * 128:(2 * kbp + 1) * 128],
             wvcs[(2 * kbp + 1) * 128:(2 * kbp + 2) * 128]],
            axis=1).reshape(128, 512))
    return np.ascontiguousarray(np.concatenate(blocks, axis=1)).astype(FP8)


def kernel(k, v, q, mask, spatial_bias, pos_k, pos_q,
           Wk, bk, Wv, bv, Wq, bq, Wpk, bpk, Wpq, bpq, Wo, bo, Wg, bg,
           **_unused):
    f32 = lambda x: np.asarray(x, np.float32)
    k, v, q, pos_k, pos_q = f32(k), f32(v), f32(q), f32(pos_k), f32(pos_q)
    spatial_bias = f32(spatial_bias)
    mask = np.asarray(mask)
    Wk, Wv, Wq, Wpk, Wpq, Wo, Wg = map(f32, (Wk, Wv, Wq, Wpk, Wpq, Wo, Wg))
    bk, bv, bq, bpk, bpq, bo, bg = map(f32, (bk, bv, bq, bpk, bpq, bo, bg))

    if not mask.all():
        return _np_reference(k, v, q, mask, spatial_bias, pos_k, pos_q,
                             Wk, bk, Wv, bv, Wq, bq, Wpk, bpk, Wpq, bpq,
                             Wo, bo, Wg, bg)

    nc = _get_nc()

    # xqr: per chunk ch: [q-side kb strip 0..7 | p-side kb strip 0..7],
    # each strip [128, 512] horizontally concatenated.
    xqr_b, xkk_b, xvr_b, sbq_b = [], [], [], []
    for b in range(B):
        qT, pT = q[b].T.astype(BF16), pos_q[b].T.astype(BF16)   # [D, L]
        chunks = []
        for ch in range(NCH):
            csl = slice(ch * 512, (ch + 1) * 512)
            chunks.append(_hstripe(qT[:, csl]))
            chunks.append(_hstripe(pT[:, csl]))
        xqr_b.append(np.ascontiguousarray(np.concatenate(chunks, axis=1)))
        xkk_b.append(np.ascontiguousarray(
            np.vstack([k[b].T, pos_k[b].T])).astype(BF16))
        vT = v[b].T.astype(BF16)
        vtiles = []
        for half in range(2):
            hsl = slice(half * 1024, (half + 1) * 1024)
            for kbp in range(4):
                vtiles.append(vT[(2 * kbp) * 128:(2 * kbp + 1) * 128, hsl])
                vtiles.append(vT[(2 * kbp + 1) * 128:(2 * kbp + 2) * 128, hsl])
        xvr_b.append(np.ascontiguousarray(np.concatenate(vtiles, axis=1)))
        sbq_b.append(np.ascontiguousarray(spatial_bias[b, 0]).astype(BF16))

    WqT, WpqT = Wq.T * SCALE, Wpq.T * SCALE
    WkT, WpkT, WvT, WoT = Wk.T, Wpk.T, Wv.T, Wo.T
    in_maps = []
    for c in range(NCORES):
        b, g = c // 4, c % 4
        cs = slice(g * CS, (g + 1) * CS)

        def chs(hl):  # global channel slice for local head hl
            h = 4 * g + hl
            return slice(h * HK, (h + 1) * HK)

        wq2_a = np.empty((D, 256), np.float32)
        wp2_a = np.empty((D, 256), np.float32)
        wkc_a = np.empty((2 * D, 256), np.float32)
        wkk_a = np.empty((D, 256), np.float32)
        pb_a = np.empty((128, 8), np.float32)
        for pr in range(2):
            he, ho = 2 * pr, 2 * pr + 1
            pc = slice(pr * 128, pr * 128 + 64)
            pc2 = slice(pr * 128 + 64, pr * 128 + 128)
            wq2_a[:, pc] = WqT[:, chs(he)]
            wq2_a[:, pc2] = WqT[:, chs(ho)]
            wp2_a[:, pc] = WpqT[:, chs(ho)]      # swapped
            wp2_a[:, pc2] = WpqT[:, chs(he)]
            wkc_a[0:D, pc] = WkT[:, chs(he)]
            wkc_a[D:, pc] = WpkT[:, chs(he)]
            wkc_a[0:D, pc2] = WkT[:, chs(ho)]
            wkc_a[D:, pc2] = WpkT[:, chs(ho)]
            wkk_a[:, pc] = WkT[:, chs(ho)]       # swapped
            wkk_a[:, pc2] = WkT[:, chs(he)]
            # per-partition biases for the half-copies
            c0 = 2 * pr
            pb_a[0:64, c0] = bq[chs(he)] * SCALE
            pb_a[64:128, c0] = bq[chs(ho)] * SCALE
            pb_a[0:64, c0 + 1] = bpq[chs(ho)] * SCALE
            pb_a[64:128, c0 + 1] = bpq[chs(he)] * SCALE
            c4 = 4 + 2 * pr
            pb_a[0:64, c4] = (bk + bpk)[chs(he)]
            pb_a[64:128, c4] = (bk + bpk)[chs(ho)]
            pb_a[0:64, c4 + 1] = bk[chs(ho)]
            pb_a[64:128, c4 + 1] = bk[chs(he)]
        # wqp stripes: per kb: [qcA | qcB | qpA(sw) | qpB(sw)] (4x128 cols)
        wqp_a = np.empty((128, 4096), np.float32)
        for kb in range(KB_D):
            rsl = slice(kb * 128, (kb + 1) * 128)
            wqp_a[:, kb * 512:kb * 512 + 256] = wq2_a[rsl, :]
            wqp_a[:, kb * 512 + 256:(kb + 1) * 512] = wp2_a[rsl, :]
        wg8_a = np.empty((128, 128), np.float32)
        wg8_a[0:64, :] = np.repeat((Wg[0] * (1.0 / SCALE))[:, None], 128, axis=1)
        wg8_a[64:128, :] = wg8_a[0:64, :]
        in_maps.append({
            "xqr": xqr_b[b], "xkk": xkk_b[b], "xvr": xvr_b[b],
            "sbq": sbq_b[b],
            "wqp": wqp_a.astype(BF16),
            "wkc2": _hstripe(wkc_a).astype(BF16),
            "wkk2": _hstripe(wkk_a).astype(BF16),
            "wvr": _hstripe(np.ascontiguousarray(WvT[:, cs])).astype(BF16),
            "wg8": wg8_a.astype(BF16),
            "wo": np.ascontiguousarray(WoT[cs, :]).astype(BF16),
            "pb": pb_a,
            "g0": np.full((128, 1), float(bg[0]), np.float32),
        })

    res = run_bass_kernel_spmd(nc, in_maps, core_ids=list(range(NCORES)))

    const_row = (bv @ WoT + bo).astype(np.float32)  # exact bv/bo fold
    out = np.empty((B, L, D), np.float32)
    for b in range(B):
        acc = res.results[b * 4]["outT"].astype(np.float32, copy=True)
        for g in range(1, 4):
            acc += res.results[b * 4 + g]["outT"]
        out[b] = acc.T + const_row
    return out
